# revision 67
# baseline (speedup 1.0000x reference)
"""Multi-head causal attention block on 8 Trainium2 NeuronCores.

Sharding: tensor-parallel over heads (4 groups of 4 heads) x data-parallel
over batch (2). Core c -> (batch b=c//4, head-group g=c%4). Each core
computes q/k/v projections for its head group, causal attention for its 4
heads, and a partial output projection; the host sums the 4 partials per
batch and adds (bo + bv @ wo^T) once (softmax rows sum to 1, so the v-bias
contributes exactly bv @ wo^T).

All matmul operands are bf16. The kernel is a single pipelined pass:
window s runs projection slice s, attention q-block s-1, and output
projection for q-block s-2 interleaved at ~850ns granularity so the PE
stream never blocks on Activation-engine exp. The softmax denominator is
built from DVE wide-folds of the exp tiles plus short accumulating
ones-matmuls (cheap on PE), not a full ones-matmul per k-tile.

Self-contained: hardcodes shapes for the 2x2048x2048, 16-head problem.
"""

from contextlib import ExitStack

import numpy as np

import concourse.bass as bass
import concourse.tile as tile
from concourse import bacc, mybir
from concourse.bass import ds, ts
from concourse.bass_utils import run_bass_kernel_spmd

F32 = mybir.dt.float32
BF16 = mybir.dt.bfloat16
ACTF = mybir.ActivationFunctionType

# Full-problem dims
BATCH = 2
SEQ = 2048
D_MODEL = 2048
NUM_HEADS = 16
HEAD_DIM = 128
N_CORES = 8
N_GROUPS = 4  # head-groups (tensor parallel)
DG = D_MODEL // N_GROUPS  # 512 = 4 heads per group
SCALE = 1.0 / float(np.sqrt(HEAD_DIM))

SL = 512  # projection slice width (seq) == attention q-block width
QB = 512
KT = 128
N_SL = SEQ // SL  # 4
N_KD = D_MODEL // 128  # 16 contraction tiles
N_DG = DG // 128  # 4 heads per group


def _interleave(primary, filler):
    """Merge unit lists: spread primary units evenly among filler units.

    Each unit is a zero-arg callable. Emits all units exactly once.
    """
    np_, nf = len(primary), len(filler)
    if np_ == 0:
        for u in filler:
            u()
        return
    if nf == 0:
        for u in primary:
            u()
        return
    fi = 0
    for i, u in enumerate(primary):
        u()
        # after primary unit i, emit filler up to proportional position
        target = (i + 1) * nf // np_
        while fi < target:
            filler[fi]()
            fi += 1
    while fi < nf:
        filler[fi]()
        fi += 1


def _mha_body(ctx, tc, aps, S, D, DGl):
    nc = tc.nc
    n_sl = S // SL
    xt, out = aps["xt"], aps["out"]
    wts = {"wv": aps["wvt"]}

    # ---------------- persistent SBUF tiles ----------------
    consts = ctx.enter_context(tc.tile_pool(name="consts", bufs=1))
    warm = consts.tile([128, 1], F32, name="act_warm")
    nc.vector.memset(warm[:], 0.0)
    nc.scalar.activation(warm[:], warm[:], ACTF.Identity, bias=warm[:, 0:1])
    # merged small constants: one f32 DMA (bq|bk) + one bf16 DMA (ones|mask)
    bqk_sb = consts.tile([128, 2 * N_DG], F32, name="bqk_sb")
    om_sb = consts.tile([128, 129], BF16, name="om_sb")

    def ones_ap():
        return om_sb[:, ds(0, 1)]

    def mask_ap():
        return om_sb[:, ds(1, 128)]

    # weights as per-DMA tiles so readers only wait on the DMA they need:
    # wk/wq as one tile per head m-block (wk m0 further split in k-halves
    # to cut the first-matmul critical path), wv as two k-half tiles
    wpool = ctx.enter_context(tc.tile_pool(name="wpool", bufs=1))
    wk_m0p = [
        wpool.tile([128, nk * 128], BF16, name=f"wk_m0{i}")
        for i, nk in enumerate((8, 8))
    ]
    _wk_m0_k0 = (0, 8)
    wk_m = [None] + [
        wpool.tile([128, N_KD * 128], BF16, name=f"wk_m{m}") for m in range(1, N_DG)
    ]
    wq_m = [wpool.tile([128, N_KD * 128], BF16, name=f"wq_m{m}") for m in range(N_DG)]
    wv_h = [wpool.tile([128, (N_KD // 2) * DGl], BF16, name=f"wv_h{i}") for i in range(2)]
    wo_sb = wpool.tile([128, N_DG * D], BF16, name="wo_sb")

    def wk_ap(m, k):
        if m == 0:
            i = 0 if k < 8 else 1
            return wk_m0p[i][:, ds((k - _wk_m0_k0[i]) * 128, 128)]
        return wk_m[m][:, ds(k * 128, 128)]

    kt_pool = ctx.enter_context(tc.tile_pool(name="kt_pool", bufs=1))
    kt_res = [kt_pool.tile([128, S], BF16, name=f"kt{h}") for h in range(N_DG)]
    v_res = [kt_pool.tile([128, DGl], BF16, name=f"v{t}") for t in range(S // 128)]
    ctx_sbs = [kt_pool.tile([128, S], BF16, name=f"ctx{h}") for h in range(N_DG)]

    xpool = ctx.enter_context(tc.tile_pool(name="xpool", bufs=2))
    qt_pool = ctx.enter_context(tc.tile_pool(name="qt_pool", bufs=2))
    lrec_pool = ctx.enter_context(tc.tile_pool(name="lrec_pool", bufs=2))
    bc_pool = ctx.enter_context(tc.tile_pool(name="bc_pool", bufs=2))
    ostage = ctx.enter_context(tc.tile_pool(name="ostage", bufs=2))

    mm_ps = ctx.enter_context(tc.tile_pool(name="mm_ps", bufs=2, space="PSUM"))
    sc_ps = ctx.enter_context(tc.tile_pool(name="sc_ps", bufs=2, space="PSUM"))
    c_ps = ctx.enter_context(tc.tile_pool(name="c_ps", bufs=2, space="PSUM"))

    # ---------------- DMA helpers ----------------
    def load_w_mblock(wname, m):
        # host provides [m, p, k*128+j] contiguous layout: 4KB runs per
        # partition (sub-512B runs pay a 2x DMA latency penalty)
        src = aps["wkp" if wname == "wk" else "wqp"]
        if wname == "wk" and m == 0:
            for i, (k0, nk) in enumerate(zip(_wk_m0_k0, (8, 8))):
                nc.sync.dma_start(wk_m0p[i][:], src[0, :, ds(k0 * 128, nk * 128)])
            return
        t = (wk_m if wname == "wk" else wq_m)[m]
        nc.sync.dma_start(t[:], src[m])

    def load_wv_khalf(hlf):
        half = N_KD // 2
        nc.sync.dma_start(
            wv_h[hlf][:].rearrange("p (k f) -> p k f", k=half),
            wts["wv"].rearrange("(k p) f -> p k f", p=128)[:, ds(hlf * half, half), :],
        )

    def load_wo():
        nc.sync.dma_start(
            wo_sb[:].rearrange("p (k f) -> p k f", k=N_DG),
            aps["wot"].rearrange("(k p) f -> p k f", p=128),
        )

    def load_xt_range(s, k0, nk, t):
        # t is a dedicated tile [128, nk*SL] covering k-tiles k0..k0+nk
        nc.sync.dma_start(
            t[:].rearrange("p (k f) -> p k f", k=nk),
            xt[ds(k0 * 128, nk * 128), ts(s, SL)].rearrange("(k p) f -> p k f", p=128),
        )

    # ---------------- projection units for slice s ----------------
    # 12 psum tiles per slice (4 k-m, 4 v-sub, 4 q-m), each 16 matmuls
    # emitted as 4 chunks of 4, plus a drain. x slice is two half tiles
    # (xa: k 0-7, xb: k 8-15) so the first chunks only depend on xa's DMA.
    def proj_units(s, xtiles, qt_sb):
        units = []

        def xs(k, off=0, w=SL):
            for t, k0, nk in xtiles:
                if k0 <= k < k0 + nk:
                    return t[:, ds((k - k0) * SL + off, w)]
            raise AssertionError(k)

        def qk_tile(wname, m, res_ap, bias_off):
            ps = mm_ps.tile([128, SL], F32, tag="mm", name="ps_qk")

            def wap(k):
                if wname == "wk":
                    return wk_ap(m, k)
                return wq_m[m][:, ds(k * 128, 128)]

            def chunk(k0, k1):
                def u():
                    for k in range(k0, k1):
                        nc.tensor.matmul(
                            ps[:],
                            lhsT=wap(k),
                            rhs=xs(k),
                            start=(k == 0),
                            stop=(k == N_KD - 1),
                        )
                return u

            for c in range(4):
                units.append(chunk(c * 4, c * 4 + 4))

            def drain():
                nc.scalar.activation(
                    res_ap, ps[:], ACTF.Identity, bias=bqk_sb[:, ds(bias_off + m, 1)]
                )

            units.append(drain)

        def v_tile(msub):
            ps = mm_ps.tile([128, DGl], F32, tag="mm", name="ps_v")

            def chunk(k0, k1):
                def u():
                    for k in range(k0, k1):
                        nc.tensor.matmul(
                            ps[:],
                            lhsT=xs(k, msub * 128, 128),
                            rhs=wv_h[k // (N_KD // 2)][:, ts(k % (N_KD // 2), DGl)],
                            start=(k == 0),
                            stop=(k == N_KD - 1),
                        )
                return u

            for c in range(4):
                units.append(chunk(c * 4, c * 4 + 4))

            def drain():
                nc.scalar.copy(v_res[s * 4 + msub][:], ps[:])

            units.append(drain)

        for m in range(N_DG):
            qk_tile("wk", m, kt_res[m][:, ts(s, SL)], N_DG)
        for msub in range(4):
            v_tile(msub)
        for m in range(N_DG):
            qk_tile("wq", m, qt_sb[:, ts(m, SL)], 0)
        return units

    # ---------------- attention units for q-block qb ----------------
    def attn_units(qb, qt_sb, ex_pool):
        units = []
        pend_tail = None  # previous head's tail_b, staggered for fold latency
        n_kt = (qb + 1) * 4
        diag0 = qb * 4
        n_pair = n_kt // 2

        for h in range(N_DG):
            ex = ex_pool.tile([128, n_kt * 512], BF16, tag="ex", name="ex")
            ps_c = c_ps.tile([128, QB], F32, tag="c", name="ps_c")
            state = {}

            def sc_of(kt, diag0=diag0):
                off = kt - diag0
                return 0 if off < 0 else off * 128

            def pair_unit(p, h=h, ex=ex, ps_c=ps_c, state=state):
                def u():
                    kts = (2 * p, 2 * p + 1)
                    is_diag = kts[0] >= diag0
                    ps_s = sc_ps.tile([128, 1024], F32, tag="s", name="ps_s")
                    for i, kt in enumerate(kts):
                        sc = sc_of(kt)
                        nc.tensor.matmul(
                            ps_s[:, ds(i * 512 + sc, 512 - sc)],
                            lhsT=kt_res[h][:, ts(kt, 128)],
                            rhs=qt_sb[:, ds(h * SL + sc, 512 - sc)],
                            start=True,
                            stop=True,
                        )
                    # PVs of previous pair (software pipeline)
                    if "prev" in state:
                        pp = state["prev"]
                        for kt in (2 * pp, 2 * pp + 1):
                            sc = sc_of(kt)
                            nc.tensor.matmul(
                                ps_c[:, ds(sc, 512 - sc)],
                                lhsT=v_res[kt][:, ts(h, 128)],
                                rhs=ex[:, ds(kt * 512 + sc, 512 - sc)],
                                start=(kt == 0),
                                stop=(kt == n_kt - 1),
                                skip_group_check=True,
                            )
                    if not is_diag:
                        # merged exp over both (fully written) halves
                        nc.scalar.activation(
                            ex[:, ds(kts[0] * 512, 1024)],
                            ps_s[:],
                            ACTF.Exp,
                            scale=SCALE,
                        )
                    else:
                        # diag tiles: individually trimmed exps (the psum
                        # region left of each sc is unwritten), then zero
                        # the triangular band via mask multiply
                        for i, kt in enumerate(kts):
                            sc = sc_of(kt)
                            nc.scalar.activation(
                                ex[:, ds(kt * 512 + sc, 512 - sc)],
                                ps_s[:, ds(i * 512 + sc, 512 - sc)],
                                ACTF.Exp,
                                scale=SCALE,
                            )
                        for i, kt in enumerate(kts):
                            off = kt - diag0
                            band = kt * 512 + off * 128
                            nc.vector.tensor_mul(
                                ex[:, ds(band, 128)],
                                ex[:, ds(band, 128)],
                                mask_ap(),
                            )
                    state["prev"] = p
                return u

            head_units = [pair_unit(p) for p in range(n_pair)]

            def tail_a(h=h, ex=ex, ps_c=ps_c, state=state):
                def u():
                    # last pair's PVs
                    pp = state["prev"]
                    for kt in (2 * pp, 2 * pp + 1):
                        sc = sc_of(kt)
                        nc.tensor.matmul(
                            ps_c[:, ds(sc, 512 - sc)],
                            lhsT=v_res[kt][:, ts(h, 128)],
                            rhs=ex[:, ds(kt * 512 + sc, 512 - sc)],
                            start=(kt == 0),
                            stop=(kt == n_kt - 1),
                            skip_group_check=True,
                        )
    # fold full sections down to <= 4 on DVE (bf16 2x mode) in
                    # place into ex sections 0..3, then fold the trimmed
                    # diagonal sections on top. All PVs for this head are
                    # done, so in-place ex edits are safe.
                    n_full = diag0
                    with nc.allow_low_precision(reason="colsum fold, <=3 roundings"):
                        if n_full == 0:
                            # qb0: fold trimmed diags onto section 0
                            for off in range(1, 4):
                                sc = off * 128
                                nc.vector.tensor_add(
                                    ex[:, ds(sc, 512 - sc)],
                                    ex[:, ds(sc, 512 - sc)],
                                    ex[:, ds(off * 512 + sc, 512 - sc)],
                                )
                            state["nsec"] = 1
                            return
                        if n_full >= 8:
                            nc.vector.tensor_add(
                                ex[:, ds(0, 2048)],
                                ex[:, ds(0, 2048)],
                                ex[:, ds(2048, 2048)],
                            )
                        if n_full == 12:
                            nc.vector.tensor_add(
                                ex[:, ds(0, 2048)],
                                ex[:, ds(0, 2048)],
                                ex[:, ds(4096, 2048)],
                            )
                        for off in range(4):
                            sc = off * 128
                            nc.vector.tensor_add(
                                ex[:, ds(off * 512 + sc, 512 - sc)],
                                ex[:, ds(off * 512 + sc, 512 - sc)],
                                ex[:, ds((diag0 + off) * 512 + sc, 512 - sc)],
                            )
                        if qb < 3:
                            # fold 4 -> 1 to save three ones-matmuls on PE
                            # (qb3 keeps 4 sections: the longer DVE chain
                            # would delay the tail's norm path)
                            nc.vector.tensor_add(
                                ex[:, ds(0, 1024)],
                                ex[:, ds(0, 1024)],
                                ex[:, ds(1024, 1024)],
                            )
                            nc.vector.tensor_add(
                                ex[:, ds(0, 512)],
                                ex[:, ds(0, 512)],
                                ex[:, ds(512, 512)],
                            )
                            state["nsec"] = 1
                        else:
                            state["nsec"] = 4
                return u

            head_units.append(tail_a())

            def tail_b(h=h, ex=ex, ps_c=ps_c, state=state, qb=qb):
                def u():
                    # denominator: accumulating ones-matmuls over the folded
                    # column-sum sections
                    lt = mm_ps.tile([1, QB], F32, tag="mm", name="ps_l")
                    n_sec = state["nsec"]
                    for j in range(n_sec):
                        nc.tensor.matmul(
                            lt[:],
                            lhsT=ones_ap(),
                            rhs=ex[:, ds(j * 512, 512)],
                            start=(j == 0),
                            stop=(j == n_sec - 1),
                            skip_group_check=True,
                        )
                    rec = lrec_pool.tile([1, QB], F32, tag="r", name="rec")
                    nc.vector.reciprocal(rec[:], lt[:])
                    bc = bc_pool.tile([128, QB], F32, tag="bc", name="bc")
                    nc.gpsimd.partition_broadcast(bc[:], rec[:])
                    with nc.allow_low_precision(reason="ctx bf16, single rounding"):
                        nc.vector.tensor_mul(
                            ctx_sbs[h][:, ts(qb, QB)], ps_c[:], bc[:]
                        )
                return u

            if pend_tail is not None:
                head_units.insert(1, pend_tail)
            pend_tail = tail_b()
            units += head_units
        units.append(pend_tail)
        return units

    # ---------------- out-proj units for q-block qb ----------------
    # one bf16 staging row-tile [128, D] per seq m-tile; 4 psum drains fill
    # its quarters, then a single DMA writes the row
    def out_units(qb, copy_engine):
        units = []
        for m in range(qb * 4, qb * 4 + 4):
            row = {}

            def mk(m=m, row=row):
                def u_alloc():
                    row["t"] = ostage.tile([128, D], BF16, tag="ot", name="ot")
                return u_alloc

            alloc = mk()
            for n in range(D // QB):
                def u(m=m, n=n, row=row, alloc=alloc):
                    if n == 0:
                        alloc()
                    ps = mm_ps.tile([128, QB], F32, tag="mm", name="ps_o")
                    for k in range(N_DG):
                        nc.tensor.matmul(
                            ps[:],
                            lhsT=ctx_sbs[k][:, ts(m, 128)],
                            rhs=wo_sb[:, ds(k * D + n * QB, QB)],
                            start=(k == 0),
                            stop=(k == N_DG - 1),
                        )
                    ot = row["t"]
                    eng = copy_engine
                    if eng == "alt":
                        eng = "act" if n % 2 == 0 else "dve"
                    if eng == "act":
                        nc.scalar.copy(ot[:, ts(n, QB)], ps[:])
                    else:
                        with nc.allow_low_precision(reason="out partial bf16"):
                            nc.vector.tensor_scalar_add(ot[:, ts(n, QB)], ps[:], 0.0)
                    if m == SEQ // 128 - 1 and n >= 2:
                        # final row: quarter DMAs to shorten the end drain
                        nc.sync.dma_start(
                            out[ts(m, 128), ts(n, QB)], ot[:, ts(n, QB)]
                        )
                    elif n % 2 == 1:
                        nc.sync.dma_start(
                            out[ts(m, 128), ds((n - 1) * QB, 2 * QB)],
                            ot[:, ds((n - 1) * QB, 2 * QB)],
                        )
                units.append(u)
        return units

    # ---------------- schedule ----------------
    xt_sbs = {}
    qt_sbs = {}

    def new_xq(s):
        ts_ = [
            (xpool.tile([128, 4 * SL], BF16, tag=f"xt{q}", name="xt_sb"), q * 4, 4)
            for q in range(4)
        ]
        xt_sbs[s] = ts_
        return ts_

    def load_x(s, xtiles):
        for t, k0, nk in xtiles:
            load_xt_range(s, k0, nk, t)

    def new_qt(s):
        t = qt_pool.tile([128, N_DG * SL], BF16, tag="qt", name="qt_sb")
        qt_sbs[s] = t
        return t

    # initial DMAs, interleaved x pieces and wk blocks so the first
    # matmuls' dependencies land earliest on the serialized DMA resource.
    # Slice 0's first k-quarter is two eighth tiles (scoped pool) and wk m0
    # is quartered: the first matmul only waits ~2 small transfers.
    x0 = new_xq(0)
    load_xt_range(0, 0, 4, x0[0][0])
    load_w_mblock("wk", 0)
    load_xt_range(0, 4, 4, x0[1][0])
    nc.sync.dma_start(bqk_sb[:], aps["bqk"])
    nc.sync.dma_start(om_sb[:], aps["om"])
    load_xt_range(0, 8, 4, x0[2][0])
    load_xt_range(0, 12, 4, x0[3][0])
    load_w_mblock("wk", 1)
    load_w_mblock("wk", 2)
    load_w_mblock("wk", 3)
    load_wv_khalf(0)
    load_wv_khalf(1)
    for m in range(N_DG):
        load_w_mblock("wq", m)

    # window 0: proj slice 0 only; prefetch x1, wo
    u = proj_units(0, x0, new_qt(0))
    load_x(1, new_xq(1))
    load_wo()
    for f in u:
        f()

    # windows 1..3: proj slice s + attn qb s-1 + out qb s-2
    copy_eng = {0: "act", 1: "dve", 2: "dve", 3: "alt"}
    for s in range(1, N_SL):
        qb = s - 1
        if s + 1 < N_SL:
            load_x(s + 1, new_xq(s + 1))
        filler = proj_units(s, xt_sbs[s], new_qt(s))
        if s >= 2:
            filler += out_units(s - 2, copy_eng[s - 2])
        with tc.tile_pool(name=f"ex{qb}", bufs=2) as ex_pool:
            primary = attn_units(qb, qt_sbs[qb], ex_pool)
            _interleave(primary, filler)

    # tail: attn qb3 + out qb2 (holding back a few units to cover the
    # last head's denominator-chain latency), then out qb3
    with tc.tile_pool(name="ex3", bufs=2) as ex_pool:
        primary = attn_units(3, qt_sbs[3], ex_pool)
        filler = out_units(2, copy_eng[2])
        _interleave(primary, filler[:-4])
        for f in filler[-4:]:
            f()
    for f in out_units(3, copy_eng[3]):
        f()


def build_program(S=SEQ, D=D_MODEL, DGl=DG, enable_asserts=False):
    nc = bacc.Bacc(
        "TRN2",
        target_bir_lowering=False,
        debug=False,
        enable_asserts=enable_asserts,
        num_devices=N_CORES,
    )
    aps = {
        "xt": nc.dram_tensor("xt", [D, S], BF16, kind="ExternalInput").ap(),
        "wqp": nc.dram_tensor(
            "wqp", [N_DG, 128, (D // 128) * 128], BF16, kind="ExternalInput"
        ).ap(),
        "wkp": nc.dram_tensor(
            "wkp", [N_DG, 128, (D // 128) * 128], BF16, kind="ExternalInput"
        ).ap(),
        "wvt": nc.dram_tensor("wvt", [D, DGl], BF16, kind="ExternalInput").ap(),
        "wot": nc.dram_tensor("wot", [DGl, D], BF16, kind="ExternalInput").ap(),
        "bqk": nc.dram_tensor("bqk", [128, 8], F32, kind="ExternalInput").ap(),
        "om": nc.dram_tensor("om", [128, 129], BF16, kind="ExternalInput").ap(),
        "out": nc.dram_tensor("out", [S, D], BF16, kind="ExternalOutput").ap(),
    }
    with tile.TileContext(nc) as tc:
        with ExitStack() as ctx:
            _mha_body(ctx, tc, aps, S, D, DGl)
    nc.compile()
    return nc


def make_om():
    """[ones | multiplicative causal band mask (1.0 where p <= j)], bf16."""
    import ml_dtypes

    p = np.arange(128)[:, None]
    j = np.arange(128)[None, :]
    om = np.ones((128, 129), np.float32)
    om[:, 1:] = (p <= j).astype(np.float32)
    return om.astype(ml_dtypes.bfloat16)


def wm_layout(w, sl):
    """Per-m-block DMA-friendly layout: wmp[m, p, k*128+j] = w[sl][m*128+j, k*128+p]."""
    import ml_dtypes

    w_sl = np.asarray(w, np.float32)[sl]  # [DG, D]
    arr = w_sl.reshape(N_DG, 128, D_MODEL // 128, 128)  # [m, j, k, p]
    return np.ascontiguousarray(arr.transpose(0, 3, 2, 1)).reshape(
        N_DG, 128, -1
    ).astype(ml_dtypes.bfloat16)


def shard_inputs(x, wq, bq, wk, bk, wv, bv, wo, bo):
    """Build the 8 per-core input maps (host-side layout prep, bf16)."""
    import ml_dtypes

    BF = ml_dtypes.bfloat16
    om = make_om()
    xts = [np.ascontiguousarray(np.asarray(x[b], np.float32).T).astype(BF) for b in range(BATCH)]
    in_maps = []
    for c in range(N_CORES):
        b, g = divmod(c, N_GROUPS)
        sl = slice(g * DG, (g + 1) * DG)
        bqk = np.empty((128, 8), np.float32)
        bqk[:, 0:4] = np.asarray(bq, np.float32)[sl].reshape(-1, 128).T
        bqk[:, 4:8] = np.asarray(bk, np.float32)[sl].reshape(-1, 128).T
        in_maps.append(
            {
                "xt": xts[b],
                "wqp": wm_layout(wq, sl),
                "wkp": wm_layout(wk, sl),
                "wvt": np.ascontiguousarray(np.asarray(wv, np.float32)[sl].T).astype(BF),
                "wot": np.ascontiguousarray(np.asarray(wo, np.float32)[:, sl].T).astype(BF),
                "bqk": bqk,
                "om": om,
            }
        )
    return in_maps


def out_bias(bv, wo, bo):
    """Host-side constant: bo + bv @ wo^T (softmax rows sum to 1)."""
    return (
        np.asarray(bo, np.float64)
        + np.asarray(bv, np.float64) @ np.asarray(wo, np.float64).T
    ).astype(np.float32)


_NC_CACHE = {}


def get_program():
    if "nc" not in _NC_CACHE:
        _NC_CACHE["nc"] = build_program()
    return _NC_CACHE["nc"]


def run_sharded(inputs, trace=False):
    nc = get_program()
    in_maps = shard_inputs(**inputs)
    res = run_bass_kernel_spmd(nc, in_maps, list(range(N_CORES)), trace=trace)
    bias = out_bias(inputs["bv"], inputs["wo"], inputs["bo"])
    full = np.empty((BATCH, SEQ, D_MODEL), np.float32)
    for b in range(BATCH):
        acc = res.results[b * N_GROUPS]["out"].astype(np.float32)
        for g in range(1, N_GROUPS):
            acc += res.results[b * N_GROUPS + g]["out"].astype(np.float32)
        full[b] = acc + bias
    return full, res


def kernel(**inputs):
    out, _ = run_sharded(inputs, trace=False)
    return out


# revision 75
# speedup vs baseline: 1.0074x; 1.0074x over previous
"""Multi-head causal attention block on 8 Trainium2 NeuronCores.

Sharding: tensor-parallel over heads (4 groups of 4 heads) x data-parallel
over batch (2). Core c -> (batch b=c//4, head-group g=c%4). Each core
computes q/k/v projections for its head group, causal attention for its 4
heads, and a partial output projection; the host sums the 4 partials per
batch and adds (bo + bv @ wo^T) once (softmax rows sum to 1, so the v-bias
contributes exactly bv @ wo^T).

All matmul operands are bf16. The kernel is a single pipelined pass:
window s runs projection slice s, attention q-block s-1, and output
projection for q-block s-2 interleaved at ~850ns granularity so the PE
stream never blocks on Activation-engine exp. The softmax denominator is
built from DVE wide-folds of the exp tiles plus short accumulating
ones-matmuls (cheap on PE), not a full ones-matmul per k-tile.

Self-contained: hardcodes shapes for the 2x2048x2048, 16-head problem.
"""

from contextlib import ExitStack

import numpy as np

import concourse.bass as bass
import concourse.tile as tile
from concourse import bacc, mybir
from concourse.bass import ds, ts
from concourse.bass_utils import run_bass_kernel_spmd

F32 = mybir.dt.float32
BF16 = mybir.dt.bfloat16
ACTF = mybir.ActivationFunctionType

# Full-problem dims
BATCH = 2
SEQ = 2048
D_MODEL = 2048
NUM_HEADS = 16
HEAD_DIM = 128
N_CORES = 8
N_GROUPS = 4  # head-groups (tensor parallel)
DG = D_MODEL // N_GROUPS  # 512 = 4 heads per group
SCALE = 1.0 / float(np.sqrt(HEAD_DIM))

SL = 512  # projection slice width (seq) == attention q-block width
QB = 512
KT = 128
N_SL = SEQ // SL  # 4
N_KD = D_MODEL // 128  # 16 contraction tiles
N_DG = DG // 128  # 4 heads per group


def _interleave(primary, filler):
    """Merge unit lists: spread primary units evenly among filler units.

    Each unit is a zero-arg callable. Emits all units exactly once.
    """
    np_, nf = len(primary), len(filler)
    if np_ == 0:
        for u in filler:
            u()
        return
    if nf == 0:
        for u in primary:
            u()
        return
    fi = 0
    for i, u in enumerate(primary):
        u()
        # after primary unit i, emit filler up to proportional position
        target = (i + 1) * nf // np_
        while fi < target:
            filler[fi]()
            fi += 1
    while fi < nf:
        filler[fi]()
        fi += 1


def _mha_body(ctx, tc, aps, S, D, DGl):
    nc = tc.nc
    n_sl = S // SL
    xt, out = aps["xt"], aps["out"]
    wts = {"wv": aps["wvt"]}

    # ---------------- persistent SBUF tiles ----------------
    consts = ctx.enter_context(tc.tile_pool(name="consts", bufs=1))
    warm = consts.tile([128, 1], F32, name="act_warm")
    nc.vector.memset(warm[:], 0.0)
    nc.scalar.activation(warm[:], warm[:], ACTF.Identity, bias=warm[:, 0:1])
    # PE p-state warm-up: dummy matmuls during the initial DMA wait so the
    # 3us ramp to full clock completes before real matmuls start (once
    # warm, short stalls don't reset the ramp)
    pe_warm = consts.tile([128, 512], BF16, name="pe_warm")
    nc.vector.memset(pe_warm[:], 0.0)
    # merged small constants: one f32 DMA (bq|bk) + one bf16 DMA (ones|mask)
    bqk_sb = consts.tile([128, 2 * N_DG], F32, name="bqk_sb")
    om_sb = consts.tile([128, 129], BF16, name="om_sb")

    def ones_ap():
        return om_sb[:, ds(0, 1)]

    def mask_ap():
        return om_sb[:, ds(1, 128)]

    # weights as per-DMA tiles so readers only wait on the DMA they need:
    # wk/wq as one tile per head m-block (wk m0 further split in k-halves
    # to cut the first-matmul critical path), wv as two k-half tiles
    wpool = ctx.enter_context(tc.tile_pool(name="wpool", bufs=1))
    wk_m0p = [
        wpool.tile([128, nk * 128], BF16, name=f"wk_m0{i}")
        for i, nk in enumerate((8, 8))
    ]
    _wk_m0_k0 = (0, 8)
    wk_m = [None] + [
        wpool.tile([128, N_KD * 128], BF16, name=f"wk_m{m}") for m in range(1, N_DG)
    ]
    wq_m = [wpool.tile([128, N_KD * 128], BF16, name=f"wq_m{m}") for m in range(N_DG)]
    wv_h = [wpool.tile([128, (N_KD // 2) * DGl], BF16, name=f"wv_h{i}") for i in range(2)]
    wo_sb = wpool.tile([128, N_DG * D], BF16, name="wo_sb")

    def wk_ap(m, k):
        if m == 0:
            i = 0 if k < 8 else 1
            return wk_m0p[i][:, ds((k - _wk_m0_k0[i]) * 128, 128)]
        return wk_m[m][:, ds(k * 128, 128)]

    kt_pool = ctx.enter_context(tc.tile_pool(name="kt_pool", bufs=1))
    kt_res = [kt_pool.tile([128, S], BF16, name=f"kt{h}") for h in range(N_DG)]
    v_res = [kt_pool.tile([128, DGl], BF16, name=f"v{t}") for t in range(S // 128)]
    ctx_sbs = [kt_pool.tile([128, S], BF16, name=f"ctx{h}") for h in range(N_DG)]

    xpool = ctx.enter_context(tc.tile_pool(name="xpool", bufs=2))
    qt_pool = ctx.enter_context(tc.tile_pool(name="qt_pool", bufs=2))
    lrec_pool = ctx.enter_context(tc.tile_pool(name="lrec_pool", bufs=2))
    bc_pool = ctx.enter_context(tc.tile_pool(name="bc_pool", bufs=2))
    ostage = ctx.enter_context(tc.tile_pool(name="ostage", bufs=2))

    mm_ps = ctx.enter_context(tc.tile_pool(name="mm_ps", bufs=2, space="PSUM"))
    sc_ps = ctx.enter_context(tc.tile_pool(name="sc_ps", bufs=2, space="PSUM"))
    c_ps = ctx.enter_context(tc.tile_pool(name="c_ps", bufs=2, space="PSUM"))

    # ---------------- DMA helpers ----------------
    def load_w_mblock(wname, m):
        # host provides [m, p, k*128+j] contiguous layout: 4KB runs per
        # partition (sub-512B runs pay a 2x DMA latency penalty)
        src = aps["wkp" if wname == "wk" else "wqp"]
        if wname == "wk" and m == 0:
            for i, (k0, nk) in enumerate(zip(_wk_m0_k0, (8, 8))):
                nc.sync.dma_start(wk_m0p[i][:], src[0, :, ds(k0 * 128, nk * 128)])
            return
        t = (wk_m if wname == "wk" else wq_m)[m]
        nc.sync.dma_start(t[:], src[m])

    def load_wv_khalf(hlf):
        half = N_KD // 2
        nc.sync.dma_start(
            wv_h[hlf][:].rearrange("p (k f) -> p k f", k=half),
            wts["wv"].rearrange("(k p) f -> p k f", p=128)[:, ds(hlf * half, half), :],
        )

    def load_wo():
        nc.sync.dma_start(
            wo_sb[:].rearrange("p (k f) -> p k f", k=N_DG),
            aps["wot"].rearrange("(k p) f -> p k f", p=128),
        )

    def load_xt_range(s, k0, nk, t):
        # t is a dedicated tile [128, nk*SL] covering k-tiles k0..k0+nk
        nc.sync.dma_start(
            t[:].rearrange("p (k f) -> p k f", k=nk),
            xt[ds(k0 * 128, nk * 128), ts(s, SL)].rearrange("(k p) f -> p k f", p=128),
        )

    # ---------------- projection units for slice s ----------------
    # 12 psum tiles per slice (4 k-m, 4 v-sub, 4 q-m), each 16 matmuls
    # emitted as 4 chunks of 4, plus a drain. x slice is two half tiles
    # (xa: k 0-7, xb: k 8-15) so the first chunks only depend on xa's DMA.
    def proj_units(s, xtiles, qt_sb, pair_first=False):
        units = []

        def xs(k, off=0, w=SL):
            for t, k0, nk in xtiles:
                if k0 <= k < k0 + nk:
                    return t[:, ds((k - k0) * SL + off, w)]
            raise AssertionError(k)

        def qk_tile(wname, m, res_ap, bias_off, chunk_sink=None):
            ps = mm_ps.tile([128, SL], F32, tag="mm", name="ps_qk")

            def wap(k):
                if wname == "wk":
                    return wk_ap(m, k)
                return wq_m[m][:, ds(k * 128, 128)]

            def chunk(k0, k1):
                def u():
                    for k in range(k0, k1):
                        nc.tensor.matmul(
                            ps[:],
                            lhsT=wap(k),
                            rhs=xs(k),
                            start=(k == 0),
                            stop=(k == N_KD - 1),
                        )
                return u

            def drain():
                nc.scalar.activation(
                    res_ap, ps[:], ACTF.Identity, bias=bqk_sb[:, ds(bias_off + m, 1)]
                )

            if chunk_sink is not None:
                chunk_sink.append([chunk(c * 4, c * 4 + 4) for c in range(4)] + [drain])
                return
            for c in range(4):
                units.append(chunk(c * 4, c * 4 + 4))
            units.append(drain)

        def v_tile(msub):
            ps = mm_ps.tile([128, DGl], F32, tag="mm", name="ps_v")

            def chunk(k0, k1):
                def u():
                    for k in range(k0, k1):
                        nc.tensor.matmul(
                            ps[:],
                            lhsT=xs(k, msub * 128, 128),
                            rhs=wv_h[k // (N_KD // 2)][:, ts(k % (N_KD // 2), DGl)],
                            start=(k == 0),
                            stop=(k == N_KD - 1),
                        )
                return u

            for c in range(4):
                units.append(chunk(c * 4, c * 4 + 4))

            def drain():
                nc.scalar.copy(v_res[s * 4 + msub][:], ps[:])

            units.append(drain)

        if pair_first:
            # slice 0: interleave the k-chunks of the first two k-proj
            # tiles so the PE consumes x-quarters as they stream in
            sink = []
            qk_tile("wk", 0, kt_res[0][:, ts(s, SL)], N_DG, chunk_sink=sink)
            qk_tile("wk", 1, kt_res[1][:, ts(s, SL)], N_DG, chunk_sink=sink)
            for c in range(4):
                units.append(sink[0][c])
                units.append(sink[1][c])
            units.append(sink[0][4])
            units.append(sink[1][4])
            rest_k = range(2, N_DG)
        else:
            rest_k = range(N_DG)
        for m in rest_k:
            qk_tile("wk", m, kt_res[m][:, ts(s, SL)], N_DG)
        for msub in range(4):
            v_tile(msub)
        for m in range(N_DG):
            qk_tile("wq", m, qt_sb[:, ts(m, SL)], 0)
        return units

    # ---------------- attention units for q-block qb ----------------
    def attn_units(qb, qt_sb, ex_pool):
        units = []
        pend_tail = None  # previous head's tail_b, staggered for fold latency
        n_kt = (qb + 1) * 4
        diag0 = qb * 4
        n_pair = n_kt // 2

        for h in range(N_DG):
            ex = ex_pool.tile([128, n_kt * 512], BF16, tag="ex", name="ex")
            ps_c = c_ps.tile([128, QB], F32, tag="c", name="ps_c")
            state = {}

            def sc_of(kt, diag0=diag0):
                off = kt - diag0
                return 0 if off < 0 else off * 128

            def pair_unit(p, h=h, ex=ex, ps_c=ps_c, state=state):
                def u():
                    kts = (2 * p, 2 * p + 1)
                    is_diag = kts[0] >= diag0
                    ps_s = sc_ps.tile([128, 1024], F32, tag="s", name="ps_s")
                    for i, kt in enumerate(kts):
                        sc = sc_of(kt)
                        nc.tensor.matmul(
                            ps_s[:, ds(i * 512 + sc, 512 - sc)],
                            lhsT=kt_res[h][:, ts(kt, 128)],
                            rhs=qt_sb[:, ds(h * SL + sc, 512 - sc)],
                            start=True,
                            stop=True,
                        )
                    # PVs of previous pair (software pipeline)
                    if "prev" in state:
                        pp = state["prev"]
                        for kt in (2 * pp, 2 * pp + 1):
                            sc = sc_of(kt)
                            nc.tensor.matmul(
                                ps_c[:, ds(sc, 512 - sc)],
                                lhsT=v_res[kt][:, ts(h, 128)],
                                rhs=ex[:, ds(kt * 512 + sc, 512 - sc)],
                                start=(kt == 0),
                                stop=(kt == n_kt - 1),
                                skip_group_check=True,
                            )
                    if not is_diag:
                        # merged exp over both (fully written) halves
                        nc.scalar.activation(
                            ex[:, ds(kts[0] * 512, 1024)],
                            ps_s[:],
                            ACTF.Exp,
                            scale=SCALE,
                        )
                    else:
                        # diag tiles: individually trimmed exps (the psum
                        # region left of each sc is unwritten), then zero
                        # the triangular band via mask multiply
                        for i, kt in enumerate(kts):
                            sc = sc_of(kt)
                            nc.scalar.activation(
                                ex[:, ds(kt * 512 + sc, 512 - sc)],
                                ps_s[:, ds(i * 512 + sc, 512 - sc)],
                                ACTF.Exp,
                                scale=SCALE,
                            )
                        for i, kt in enumerate(kts):
                            off = kt - diag0
                            band = kt * 512 + off * 128
                            nc.vector.tensor_mul(
                                ex[:, ds(band, 128)],
                                ex[:, ds(band, 128)],
                                mask_ap(),
                            )
                    state["prev"] = p
                return u

            head_units = [pair_unit(p) for p in range(n_pair)]

            def tail_a(h=h, ex=ex, ps_c=ps_c, state=state):
                def u():
                    # last pair's PVs
                    pp = state["prev"]
                    for kt in (2 * pp, 2 * pp + 1):
                        sc = sc_of(kt)
                        nc.tensor.matmul(
                            ps_c[:, ds(sc, 512 - sc)],
                            lhsT=v_res[kt][:, ts(h, 128)],
                            rhs=ex[:, ds(kt * 512 + sc, 512 - sc)],
                            start=(kt == 0),
                            stop=(kt == n_kt - 1),
                            skip_group_check=True,
                        )
    # fold full sections down to <= 4 on DVE (bf16 2x mode) in
                    # place into ex sections 0..3, then fold the trimmed
                    # diagonal sections on top. All PVs for this head are
                    # done, so in-place ex edits are safe.
                    n_full = diag0
                    with nc.allow_low_precision(reason="colsum fold, <=3 roundings"):
                        if n_full == 0:
                            # qb0: fold trimmed diags onto section 0
                            for off in range(1, 4):
                                sc = off * 128
                                nc.vector.tensor_add(
                                    ex[:, ds(sc, 512 - sc)],
                                    ex[:, ds(sc, 512 - sc)],
                                    ex[:, ds(off * 512 + sc, 512 - sc)],
                                )
                            state["nsec"] = 1
                            return
                        if n_full >= 8:
                            nc.vector.tensor_add(
                                ex[:, ds(0, 2048)],
                                ex[:, ds(0, 2048)],
                                ex[:, ds(2048, 2048)],
                            )
                        if n_full == 12:
                            nc.vector.tensor_add(
                                ex[:, ds(0, 2048)],
                                ex[:, ds(0, 2048)],
                                ex[:, ds(4096, 2048)],
                            )
                        for off in range(4):
                            sc = off * 128
                            nc.vector.tensor_add(
                                ex[:, ds(off * 512 + sc, 512 - sc)],
                                ex[:, ds(off * 512 + sc, 512 - sc)],
                                ex[:, ds((diag0 + off) * 512 + sc, 512 - sc)],
                            )
                        if qb < 3:
                            # fold 4 -> 1 to save three ones-matmuls on PE
                            # (qb3 keeps 4 sections: the longer DVE chain
                            # would delay the tail's norm path)
                            nc.vector.tensor_add(
                                ex[:, ds(0, 1024)],
                                ex[:, ds(0, 1024)],
                                ex[:, ds(1024, 1024)],
                            )
                            nc.vector.tensor_add(
                                ex[:, ds(0, 512)],
                                ex[:, ds(0, 512)],
                                ex[:, ds(512, 512)],
                            )
                            state["nsec"] = 1
                        else:
                            state["nsec"] = 4
                return u

            head_units.append(tail_a())

            def tail_b(h=h, ex=ex, ps_c=ps_c, state=state, qb=qb):
                def u():
                    # denominator: accumulating ones-matmuls over the folded
                    # column-sum sections
                    lt = mm_ps.tile([1, QB], F32, tag="mm", name="ps_l")
                    n_sec = state["nsec"]
                    for j in range(n_sec):
                        nc.tensor.matmul(
                            lt[:],
                            lhsT=ones_ap(),
                            rhs=ex[:, ds(j * 512, 512)],
                            start=(j == 0),
                            stop=(j == n_sec - 1),
                            skip_group_check=True,
                        )
                    rec = lrec_pool.tile([1, QB], F32, tag="r", name="rec")
                    nc.vector.reciprocal(rec[:], lt[:])
                    bc = bc_pool.tile([128, QB], F32, tag="bc", name="bc")
                    nc.gpsimd.partition_broadcast(bc[:], rec[:])
                    with nc.allow_low_precision(reason="ctx bf16, single rounding"):
                        nc.vector.tensor_mul(
                            ctx_sbs[h][:, ts(qb, QB)], ps_c[:], bc[:]
                        )
                return u

            if pend_tail is not None:
                head_units.insert(1, pend_tail)
            pend_tail = tail_b()
            units += head_units
        units.append(pend_tail)
        return units

    # ---------------- out-proj units for q-block qb ----------------
    # one bf16 staging row-tile [128, D] per seq m-tile; 4 psum drains fill
    # its quarters, then a single DMA writes the row
    def out_units(qb, copy_engine):
        units = []
        for m in range(qb * 4, qb * 4 + 4):
            row = {}

            def mk(m=m, row=row):
                def u_alloc():
                    row["t"] = ostage.tile([128, D], BF16, tag="ot", name="ot")
                return u_alloc

            alloc = mk()
            for n in range(D // QB):
                def u(m=m, n=n, row=row, alloc=alloc):
                    if n == 0:
                        alloc()
                    ps = mm_ps.tile([128, QB], F32, tag="mm", name="ps_o")
                    for k in range(N_DG):
                        nc.tensor.matmul(
                            ps[:],
                            lhsT=ctx_sbs[k][:, ts(m, 128)],
                            rhs=wo_sb[:, ds(k * D + n * QB, QB)],
                            start=(k == 0),
                            stop=(k == N_DG - 1),
                        )
                    ot = row["t"]
                    eng = copy_engine
                    if eng == "alt":
                        eng = "act" if n % 2 == 0 else "dve"
                    if eng == "act":
                        nc.scalar.copy(ot[:, ts(n, QB)], ps[:])
                    else:
                        with nc.allow_low_precision(reason="out partial bf16"):
                            nc.vector.tensor_scalar_add(ot[:, ts(n, QB)], ps[:], 0.0)
                    if m == SEQ // 128 - 1 and n >= 2:
                        # final row: quarter DMAs to shorten the end drain
                        nc.sync.dma_start(
                            out[ts(m, 128), ts(n, QB)], ot[:, ts(n, QB)]
                        )
                    elif n % 2 == 1:
                        nc.sync.dma_start(
                            out[ts(m, 128), ds((n - 1) * QB, 2 * QB)],
                            ot[:, ds((n - 1) * QB, 2 * QB)],
                        )
                units.append(u)
        return units

    # ---------------- schedule ----------------
    xt_sbs = {}
    qt_sbs = {}

    def new_xq(s):
        ts_ = [
            (xpool.tile([128, 4 * SL], BF16, tag=f"xt{q}", name="xt_sb"), q * 4, 4)
            for q in range(4)
        ]
        xt_sbs[s] = ts_
        return ts_

    def load_x(s, xtiles):
        for t, k0, nk in xtiles:
            load_xt_range(s, k0, nk, t)

    def new_qt(s):
        t = qt_pool.tile([128, N_DG * SL], BF16, tag="qt", name="qt_sb")
        qt_sbs[s] = t
        return t

    # initial DMAs, interleaved x pieces and wk blocks so the first
    # matmuls' dependencies land earliest on the serialized DMA resource.
    # Slice 0's first k-quarter is two eighth tiles (scoped pool) and wk m0
    # is quartered: the first matmul only waits ~2 small transfers.
    x0 = new_xq(0)
    load_xt_range(0, 0, 4, x0[0][0])
    nc.sync.dma_start(wk_m0p[0][:], aps["wkp"][0, :, ds(0, 1024)])
    load_w_mblock("wk", 1)
    nc.sync.dma_start(bqk_sb[:], aps["bqk"])
    nc.sync.dma_start(om_sb[:], aps["om"])
    load_xt_range(0, 4, 4, x0[1][0])
    nc.sync.dma_start(wk_m0p[1][:], aps["wkp"][0, :, ds(1024, 1024)])
    load_xt_range(0, 8, 4, x0[2][0])
    load_xt_range(0, 12, 4, x0[3][0])
    load_w_mblock("wk", 2)
    load_w_mblock("wk", 3)
    load_wv_khalf(0)
    load_wv_khalf(1)
    for m in range(N_DG):
        load_w_mblock("wq", m)

    for _ in range(7):
        psw = mm_ps.tile([128, 512], F32, tag="mm", name="ps_warm")
        nc.tensor.matmul(
            psw[:], lhsT=pe_warm[:, ds(0, 128)], rhs=pe_warm[:], start=True, stop=True
        )

    # window 0: proj slice 0 only; prefetch x1, wo
    u = proj_units(0, x0, new_qt(0), pair_first=True)
    load_x(1, new_xq(1))
    load_wo()
    for f in u:
        f()

    # windows 1..3: proj slice s + attn qb s-1 + out qb s-2
    copy_eng = {0: "act", 1: "dve", 2: "dve", 3: "alt"}
    for s in range(1, N_SL):
        qb = s - 1
        if s + 1 < N_SL:
            load_x(s + 1, new_xq(s + 1))
        filler = proj_units(s, xt_sbs[s], new_qt(s))
        if s >= 2:
            filler += out_units(s - 2, copy_eng[s - 2])
        with tc.tile_pool(name=f"ex{qb}", bufs=2) as ex_pool:
            primary = attn_units(qb, qt_sbs[qb], ex_pool)
            _interleave(primary, filler)

    # tail: attn qb3 + out qb2 (holding back a few units to cover the
    # last head's denominator-chain latency), then out qb3
    with tc.tile_pool(name="ex3", bufs=2) as ex_pool:
        primary = attn_units(3, qt_sbs[3], ex_pool)
        filler = out_units(2, copy_eng[2])
        _interleave(primary, filler[:-4])
        for f in filler[-4:]:
            f()
    for f in out_units(3, copy_eng[3]):
        f()


def build_program(S=SEQ, D=D_MODEL, DGl=DG, enable_asserts=False):
    nc = bacc.Bacc(
        "TRN2",
        target_bir_lowering=False,
        debug=False,
        enable_asserts=enable_asserts,
        num_devices=N_CORES,
    )
    aps = {
        "xt": nc.dram_tensor("xt", [D, S], BF16, kind="ExternalInput").ap(),
        "wqp": nc.dram_tensor(
            "wqp", [N_DG, 128, (D // 128) * 128], BF16, kind="ExternalInput"
        ).ap(),
        "wkp": nc.dram_tensor(
            "wkp", [N_DG, 128, (D // 128) * 128], BF16, kind="ExternalInput"
        ).ap(),
        "wvt": nc.dram_tensor("wvt", [D, DGl], BF16, kind="ExternalInput").ap(),
        "wot": nc.dram_tensor("wot", [DGl, D], BF16, kind="ExternalInput").ap(),
        "bqk": nc.dram_tensor("bqk", [128, 8], F32, kind="ExternalInput").ap(),
        "om": nc.dram_tensor("om", [128, 129], BF16, kind="ExternalInput").ap(),
        "out": nc.dram_tensor("out", [S, D], BF16, kind="ExternalOutput").ap(),
    }
    with tile.TileContext(nc) as tc:
        with ExitStack() as ctx:
            _mha_body(ctx, tc, aps, S, D, DGl)
    nc.compile()
    return nc


def make_om():
    """[ones | multiplicative causal band mask (1.0 where p <= j)], bf16."""
    import ml_dtypes

    p = np.arange(128)[:, None]
    j = np.arange(128)[None, :]
    om = np.ones((128, 129), np.float32)
    om[:, 1:] = (p <= j).astype(np.float32)
    return om.astype(ml_dtypes.bfloat16)


def wm_layout(w, sl):
    """Per-m-block DMA-friendly layout: wmp[m, p, k*128+j] = w[sl][m*128+j, k*128+p]."""
    import ml_dtypes

    w_sl = np.asarray(w, np.float32)[sl]  # [DG, D]
    arr = w_sl.reshape(N_DG, 128, D_MODEL // 128, 128)  # [m, j, k, p]
    return np.ascontiguousarray(arr.transpose(0, 3, 2, 1)).reshape(
        N_DG, 128, -1
    ).astype(ml_dtypes.bfloat16)


def shard_inputs(x, wq, bq, wk, bk, wv, bv, wo, bo):
    """Build the 8 per-core input maps (host-side layout prep, bf16)."""
    import ml_dtypes

    BF = ml_dtypes.bfloat16
    om = make_om()
    xts = [np.ascontiguousarray(np.asarray(x[b], np.float32).T).astype(BF) for b in range(BATCH)]
    in_maps = []
    for c in range(N_CORES):
        b, g = divmod(c, N_GROUPS)
        sl = slice(g * DG, (g + 1) * DG)
        bqk = np.empty((128, 8), np.float32)
        bqk[:, 0:4] = np.asarray(bq, np.float32)[sl].reshape(-1, 128).T
        bqk[:, 4:8] = np.asarray(bk, np.float32)[sl].reshape(-1, 128).T
        in_maps.append(
            {
                "xt": xts[b],
                "wqp": wm_layout(wq, sl),
                "wkp": wm_layout(wk, sl),
                "wvt": np.ascontiguousarray(np.asarray(wv, np.float32)[sl].T).astype(BF),
                "wot": np.ascontiguousarray(np.asarray(wo, np.float32)[:, sl].T).astype(BF),
                "bqk": bqk,
                "om": om,
            }
        )
    return in_maps


def out_bias(bv, wo, bo):
    """Host-side constant: bo + bv @ wo^T (softmax rows sum to 1)."""
    return (
        np.asarray(bo, np.float64)
        + np.asarray(bv, np.float64) @ np.asarray(wo, np.float64).T
    ).astype(np.float32)


_NC_CACHE = {}


def get_program():
    if "nc" not in _NC_CACHE:
        _NC_CACHE["nc"] = build_program()
    return _NC_CACHE["nc"]


def run_sharded(inputs, trace=False):
    nc = get_program()
    in_maps = shard_inputs(**inputs)
    res = run_bass_kernel_spmd(nc, in_maps, list(range(N_CORES)), trace=trace)
    bias = out_bias(inputs["bv"], inputs["wo"], inputs["bo"])
    full = np.empty((BATCH, SEQ, D_MODEL), np.float32)
    for b in range(BATCH):
        acc = res.results[b * N_GROUPS]["out"].astype(np.float32)
        for g in range(1, N_GROUPS):
            acc += res.results[b * N_GROUPS + g]["out"].astype(np.float32)
        full[b] = acc + bias
    return full, res


def kernel(**inputs):
    out, _ = run_sharded(inputs, trace=False)
    return out
